# revision 2
# baseline (speedup 1.0000x reference)
"""LogicLayer Trainium2 kernel.

out[b, n] = sum_k softmax(w[n])_k * gate_k(a1, a2),  a1 = x[b, i1[n]], a2 = x[b, i2[n]]

All 16 differentiable gates are affine in {1, a1, a2, a1*a2}:
    out[b, n] = A0[n] + A1[n]*a1 + A2[n]*a2 + Ap[n]*a1*a2
with A* = softmax(w[n]) @ C for a constant [16, 4] table C.

Device plan (8 NeuronCores, neuron-sharded: 1024 neurons x full 2048 batch each):
  - x is shipped transposed AND cast to f16 (xt [8192, 2048] f16) so one
    neuron's input column is a contiguous 4KB row in HBM. The correctness
    gate is rel_err < 2e-2; f16 gathers + f16 output writes land at ~3e-4,
    while halving HBM traffic vs f32 (~12MB/core instead of ~24MB).
  - per 128-neuron slot, one gpsimd.indirect_dma_start pulls 128 rows (one per
    partition, per-partition int32 row offsets) straight from HBM into SBUF.
  - coefficients A0..Ap are computed on-device from w (ACT exp + DVE reduces).
  - inner loop is 2 fused DVE ops per 128-neuron slot (f16 data hits the
    2x-packed DVE mode):
        t   = (Ap*g2 + A1) * g1          (affine_mul_reduce)
        out = (A2*g2 + A0) + t           (affine_then_add)
  - output is written neuron-major [1024, 2048] f16; host reassembles,
    transposes and upcasts to f32.
"""

import numpy as np

BATCH = 2048
NIN = 8192
NNEUR = 8192
NCORES = 8
NN = NNEUR // NCORES  # neurons per core (1024)
NB = BATCH            # full batch per core
SLOTS = NN // 128     # 8
CHUNKS = [2, 2, 2, 1, 1]  # slots per pipeline chunk (tapered tail)

# gate -> (c0, c1, c2, cp) so gate_k(a1,a2) = c0 + c1*a1 + c2*a2 + cp*a1*a2
GATE_COEF = np.array(
    [
        [0, 0, 0, 0],    # FALSE
        [0, 0, 0, 1],    # AND
        [0, 1, 0, -1],   # a1 AND NOT a2
        [0, 1, 0, 0],    # a1
        [0, 0, 1, -1],   # NOT a1 AND a2
        [0, 0, 1, 0],    # a2
        [0, 1, 1, -2],   # XOR
        [0, 1, 1, -1],   # OR
        [1, -1, -1, 1],  # NOR
        [1, -1, -1, 2],  # XNOR
        [1, 0, -1, 0],   # NOT a2
        [1, 0, -1, 1],   # a1 OR NOT a2
        [1, -1, 0, 0],   # NOT a1
        [1, -1, 0, 1],   # NOT a1 OR a2
        [1, 0, 0, -1],   # NAND
        [1, 0, 0, 0],    # TRUE
    ],
    dtype=np.float32,
)  # [16, 4]

_CACHE = {}


def _build_nc():
    import concourse.bacc as bacc
    import concourse.bass as bass
    import concourse.mybir as mybir
    from concourse.tile import TileContext

    f32 = mybir.dt.float32
    f16 = mybir.dt.float16
    i32 = mybir.dt.int32

    nc = bacc.Bacc("TRN2")
    xt = nc.dram_tensor("xt", [NIN, NB], f16, kind="ExternalInput")
    idx1 = nc.dram_tensor("idx1", [128, SLOTS], i32, kind="ExternalInput")
    idx2 = nc.dram_tensor("idx2", [128, SLOTS], i32, kind="ExternalInput")
    wr = nc.dram_tensor("wr", [128, SLOTS * 16], f32, kind="ExternalInput")
    ctab = nc.dram_tensor("ctab", [128, 4, SLOTS * 16], f32, kind="ExternalInput")
    yt = nc.dram_tensor("yt", [NN, NB], f16, kind="ExternalOutput")

    with TileContext(nc) as tc:
        with (
            tc.tile_pool(name="coef", bufs=1) as coef_pool,
            tc.tile_pool(name="work", bufs=3) as work_pool,
            tc.tile_pool(name="outp", bufs=3) as out_pool,
        ):
            # index tiles first, on gpsimd's own SWDGE queue so the
            # gathers don't wait on a cross-engine HWDGE completion
            it1 = coef_pool.tile([128, SLOTS], i32)
            nc.gpsimd.dma_start(it1[:], idx1[:])
            it2 = coef_pool.tile([128, SLOTS], i32)
            nc.gpsimd.dma_start(it2[:], idx2[:])

            # --- coefficients A0..Ap [128, SLOTS] from w ---
            wt = coef_pool.tile([128, SLOTS * 16], f32)
            nc.sync.dma_start(wt[:], wr[:])
            ct = coef_pool.tile([128, 4, SLOTS * 16], f32)
            nc.sync.dma_start(ct[:], ctab[:])

            ew = coef_pool.tile([128, SLOTS * 16], f32)
            zb = coef_pool.tile([128, 1], f32)
            nc.vector.memset(zb[:], 0.0)
            nc.scalar.activation(
                ew[:], wt[:], mybir.ActivationFunctionType.Exp, bias=zb[:, 0:1]
            )
            ssum = coef_pool.tile([128, SLOTS], f32)
            nc.vector.tensor_reduce(
                ssum[:],
                ew[:].rearrange("p (s k) -> p s k", k=16),
                mybir.AxisListType.X,
                mybir.AluOpType.add,
            )
            rsum = coef_pool.tile([128, SLOTS], f32)
            nc.vector.reciprocal(rsum[:], ssum[:])

            acoef = coef_pool.tile([128, 4, SLOTS], f32)
            wtmp = coef_pool.tile([128, SLOTS * 16], f32)
            for c in range(4):
                nc.vector.tensor_mul(wtmp[:], ew[:], ct[:, c, :])
                nc.vector.tensor_reduce(
                    acoef[:, c, :],
                    wtmp[:].rearrange("p (s k) -> p s k", k=16),
                    mybir.AxisListType.X,
                    mybir.AluOpType.add,
                )
                nc.vector.tensor_mul(acoef[:, c, :], acoef[:, c, :], rsum[:])

            accum = coef_pool.tile([128, 1], f32)

            # --- main gather + combine loop (tapered chunks) ---
            s0 = 0
            for csl in CHUNKS:
                g1 = work_pool.tile([128, csl, NB], f16, tag="g1")
                for s in range(csl):
                    nc.gpsimd.indirect_dma_start(
                        out=g1[:, s, :], out_offset=None, in_=xt[:],
                        in_offset=bass.IndirectOffsetOnAxis(
                            ap=it1[:, s0 + s:s0 + s + 1], axis=0),
                    )
                g2 = work_pool.tile([128, csl, NB], f16, tag="g2")
                for s in range(csl):
                    nc.gpsimd.indirect_dma_start(
                        out=g2[:, s, :], out_offset=None, in_=xt[:],
                        in_offset=bass.IndirectOffsetOnAxis(
                            ap=it2[:, s0 + s:s0 + s + 1], axis=0),
                    )
                ot = out_pool.tile([128, csl, NB], f16, tag="ot")
                for s in range(csl):
                    S = s0 + s
                    # t = (Ap*g2 + A1) * g1
                    nc.vector.affine_mul_reduce(
                        ot[:, s, :], accum[:],
                        g2[:, s, :], g1[:, s, :],
                        acoef[:, 3, S:S + 1], acoef[:, 1, S:S + 1],
                    )
                    # out = (A2*g2 + A0) + t
                    nc.vector.affine_then_add(
                        ot[:, s, :],
                        g2[:, s, :], ot[:, s, :],
                        acoef[:, 2, S:S + 1], acoef[:, 0, S:S + 1],
                    )
                dst = yt[s0 * 128:(s0 + csl) * 128, :].rearrange(
                    "(s p) b -> p s b", p=128
                )
                nc.sync.dma_start(dst, ot[:])
                s0 += csl

    nc.compile()
    return nc


def _prep_core_inputs(x, w, conn_indices):
    """Host-side shard/layout prep. Returns list of per-core input dicts."""
    xt = np.ascontiguousarray(x.T.astype(np.float16))  # [NIN, BATCH] f16, shared
    ctab = np.ascontiguousarray(
        np.broadcast_to(
            GATE_COEF.T.reshape(1, 4, 1, 16), (128, 4, SLOTS, 16)
        ).reshape(128, 4, SLOTS * 16)
    )
    maps = []
    for c in range(NCORES):
        n0 = c * NN
        # neuron n0 + s*128 + p -> partition p, slot s; idx[p, s] = row offset
        core_maps = {"xt": xt, "ctab": ctab}
        for nm, col in (("idx1", 0), ("idx2", 1)):
            arr = conn_indices[n0:n0 + NN, col].reshape(SLOTS, 128).T
            core_maps[nm] = np.ascontiguousarray(arr).astype(np.int32)
        wslice = w[n0:n0 + NN, :].reshape(SLOTS, 128, 16).transpose(1, 0, 2)
        core_maps["wr"] = np.ascontiguousarray(wslice.reshape(128, SLOTS * 16))
        maps.append(core_maps)
    return maps


def run_cores(in_maps, trace=False):
    from concourse.bass_utils import run_bass_kernel_spmd

    if "nc" not in _CACHE:
        _CACHE["nc"] = _build_nc()
    return run_bass_kernel_spmd(
        _CACHE["nc"], in_maps, core_ids=list(range(NCORES)), trace=trace
    )


def _assemble(results):
    out = np.empty((BATCH, NNEUR), dtype=np.float32)
    for c in range(NCORES):
        n0 = c * NN
        out[:, n0:n0 + NN] = results[c]["yt"].T.astype(np.float32)
    return out


def kernel(x, w, conn_indices):
    x = np.asarray(x, dtype=np.float32)
    w = np.asarray(w, dtype=np.float32)
    conn_indices = np.asarray(conn_indices)
    in_maps = _prep_core_inputs(x, w, conn_indices)
    res = run_cores(in_maps)
    return _assemble([r for r in res.results])


# revision 8
# speedup vs baseline: 1.4557x; 1.4557x over previous
"""LogicLayer Trainium2 kernel.

out[b, n] = sum_k softmax(w[n])_k * gate_k(a1, a2),  a1 = x[b, i1[n]], a2 = x[b, i2[n]]

All 16 differentiable gates are affine in {1, a1, a2, a1*a2}:
    out[b, n] = A0[n] + A1[n]*a1 + A2[n]*a2 + Ap[n]*a1*a2
with A* = softmax(w[n]) @ C for a constant [16, 4] table C. A* is tiny
([8192, 4] total) and is precomputed on the host, so the device kernel is a
pure gather + 2-DVE-ops-per-slot + write pipeline.

Device plan (8 NeuronCores, neuron-sharded: 1024 neurons x full 2048 batch each):
  - x is shipped transposed and cast to f16 (xt [8192, 2048]) so one neuron's
    input column is a contiguous 4KB row in HBM. The correctness gate is
    rel_err < 2e-2; f16 gathers + f16 output writes land at ~3e-4 while
    halving HBM traffic vs f32 (~12MB/core instead of ~24MB).
  - gathers: 16 single-offset gpsimd.indirect_dma_start calls (multi-offset
    offset-APs and >32KB dest offsets within one dest tile both break the
    real SWDGE descriptor generator, although CoreSim accepts them), each
    into its own small per-slot tile.
  - all working tiles are static f16 SBUF residents (~100KB/partition), so
    there is no pool-buffer recycling and no resulting Q7 stalls.
  - inner loop is 2 fused DVE ops per 128-neuron slot (f16 data hits the
    2x-packed DVE mode):
        t   = (Ap*g2 + A1) * g1          (affine_mul_reduce)
        out = (A2*g2 + A0) + t           (affine_then_add)
  - output is written neuron-major [1024, 2048] f16, one write per slot so
    writes overlap the remaining gathers; host reassembles/transposes/upcasts.
"""

import numpy as np

BATCH = 2048
NIN = 8192
NNEUR = 8192
NCORES = 8
NN = NNEUR // NCORES  # neurons per core (1024)
NB = BATCH            # full batch per core
SLOTS = NN // 128     # 8


# gate -> (c0, c1, c2, cp) so gate_k(a1,a2) = c0 + c1*a1 + c2*a2 + cp*a1*a2
GATE_COEF = np.array(
    [
        [0, 0, 0, 0],    # FALSE
        [0, 0, 0, 1],    # AND
        [0, 1, 0, -1],   # a1 AND NOT a2
        [0, 1, 0, 0],    # a1
        [0, 0, 1, -1],   # NOT a1 AND a2
        [0, 0, 1, 0],    # a2
        [0, 1, 1, -2],   # XOR
        [0, 1, 1, -1],   # OR
        [1, -1, -1, 1],  # NOR
        [1, -1, -1, 2],  # XNOR
        [1, 0, -1, 0],   # NOT a2
        [1, 0, -1, 1],   # a1 OR NOT a2
        [1, -1, 0, 0],   # NOT a1
        [1, -1, 0, 1],   # NOT a1 OR a2
        [1, 0, 0, -1],   # NAND
        [1, 0, 0, 0],    # TRUE
    ],
    dtype=np.float32,
)  # [16, 4]

_CACHE = {}


def _build_nc():
    import concourse.bacc as bacc
    import concourse.bass as bass
    import concourse.mybir as mybir
    from concourse.tile import TileContext

    f32 = mybir.dt.float32
    f16 = mybir.dt.float16
    i32 = mybir.dt.int32

    nc = bacc.Bacc("TRN2")
    xt = nc.dram_tensor("xt", [NIN, NB], f16, kind="ExternalInput")
    # io[p, 2*s+o] = row index of operand o for neuron (slot s, partition p)
    io = nc.dram_tensor("io", [128, SLOTS * 2], i32, kind="ExternalInput")
    # ac[p, c, s] = coefficient A_c for neuron (slot s, partition p)
    ac = nc.dram_tensor("ac", [128, 4, SLOTS], f32, kind="ExternalInput")
    yt = nc.dram_tensor("yt", [NN, NB], f16, kind="ExternalOutput")

    with TileContext(nc) as tc:
        with tc.tile_pool(name="all", bufs=1) as pool:
            it = pool.tile([128, SLOTS * 2], i32)
            nc.sync.dma_start(it[:], io[:])
            act = pool.tile([128, 4, SLOTS], f32)
            nc.sync.dma_start(act[:], ac[:])

            # one small tile per slot: the SWDGE descriptor generator mangles
            # dest offsets beyond ~32KB within a single dest AP/tile, so each
            # indirect call must target its own tile at offset ~0.
            # gs[s][p, o, :] = xt[io[p, 2*s+o], :]
            gs = [pool.tile([128, 2, NB], f16, name=f"g{s}") for s in range(SLOTS)]
            ot = pool.tile([128, SLOTS, NB], f16)
            accum = pool.tile([128, 1], f32)

            for s in range(SLOTS):
                for o in range(2):
                    j = 2 * s + o
                    nc.gpsimd.indirect_dma_start(
                        out=gs[s][:, o, :], out_offset=None,
                        in_=xt[:],
                        in_offset=bass.IndirectOffsetOnAxis(
                            ap=it[:, j:j + 1], axis=0),
                    )

            for s in range(SLOTS):
                g1 = gs[s][:, 0, :]
                g2 = gs[s][:, 1, :]
                # t = (Ap*g2 + A1) * g1
                nc.vector.affine_mul_reduce(
                    ot[:, s, :], accum[:],
                    g2, g1,
                    act[:, 3, s:s + 1], act[:, 1, s:s + 1],
                )
                # out = (A2*g2 + A0) + t
                nc.vector.affine_then_add(
                    ot[:, s, :],
                    g2, ot[:, s, :],
                    act[:, 2, s:s + 1], act[:, 0, s:s + 1],
                )
                nc.sync.dma_start(yt[s * 128:(s + 1) * 128, :], ot[:, s, :])

    nc.compile()
    return nc


def _prep_core_inputs(x, w, conn_indices):
    """Host-side shard/layout prep. Returns list of per-core input dicts."""
    xt = np.ascontiguousarray(x.T.astype(np.float16))  # [NIN, BATCH] f16, shared
    # A = softmax(w) @ GATE_COEF, [NNEUR, 4] — tiny; compute on host in f64
    ew = np.exp(w.astype(np.float64))
    probs = ew / ew.sum(axis=1, keepdims=True)
    A = (probs @ GATE_COEF.astype(np.float64)).astype(np.float32)
    maps = []
    for c in range(NCORES):
        n0 = c * NN
        # neuron n0 + s*128 + p -> partition p, slot s
        idx = conn_indices[n0:n0 + NN, :].reshape(SLOTS, 128, 2)
        io = idx.transpose(1, 0, 2).reshape(128, SLOTS * 2)
        ac = A[n0:n0 + NN, :].reshape(SLOTS, 128, 4).transpose(1, 2, 0)
        maps.append({
            "xt": xt,
            "io": np.ascontiguousarray(io).astype(np.int32),
            "ac": np.ascontiguousarray(ac),
        })
    return maps


def run_cores(in_maps, trace=False):
    from concourse.bass_utils import run_bass_kernel_spmd

    if "nc" not in _CACHE:
        _CACHE["nc"] = _build_nc()
    return run_bass_kernel_spmd(
        _CACHE["nc"], in_maps, core_ids=list(range(NCORES)), trace=trace
    )


def _assemble(results):
    out = np.empty((BATCH, NNEUR), dtype=np.float32)
    for c in range(NCORES):
        n0 = c * NN
        out[:, n0:n0 + NN] = results[c]["yt"].T.astype(np.float32)
    return out


def kernel(x, w, conn_indices):
    x = np.asarray(x, dtype=np.float32)
    w = np.asarray(w, dtype=np.float32)
    conn_indices = np.asarray(conn_indices)
    in_maps = _prep_core_inputs(x, w, conn_indices)
    res = run_cores(in_maps)
    return _assemble([r for r in res.results])


# revision 10
# speedup vs baseline: 1.5044x; 1.0334x over previous
"""LogicLayer Trainium2 kernel.

out[b, n] = sum_k softmax(w[n])_k * gate_k(a1, a2),  a1 = x[b, i1[n]], a2 = x[b, i2[n]]

All 16 differentiable gates are affine in {1, a1, a2, a1*a2}:
    out[b, n] = A0[n] + A1[n]*a1 + A2[n]*a2 + Ap[n]*a1*a2
with A* = softmax(w[n]) @ C for a constant [16, 4] table C. A* is tiny
([8192, 4] total) and is precomputed on the host, so the device kernel is a
pure gather + 2-DVE-ops-per-slot + write pipeline.

Device plan (8 NeuronCores, neuron-sharded: 1024 neurons x full 2048 batch each):
  - x is shipped transposed and cast to f16 (xt [8192, 2048]) so one neuron's
    input column is a contiguous 4KB row in HBM. The correctness gate is
    rel_err < 2e-2; f16 gathers + f16 output writes land at ~3e-4 while
    halving HBM traffic vs f32 (~12MB/core instead of ~24MB).
  - gathers: 16 single-offset gpsimd.indirect_dma_start calls (multi-offset
    offset-APs and >32KB dest offsets within one dest tile both break the
    real SWDGE descriptor generator, although CoreSim accepts them), each
    into its own small per-slot tile.
  - all working tiles are static f16 SBUF residents (~100KB/partition), so
    there is no pool-buffer recycling and no resulting Q7 stalls.
  - inner loop is 2 fused DVE ops per 128-neuron slot (f16 data hits the
    2x-packed DVE mode):
        t   = (Ap*g2 + A1) * g1          (affine_mul_reduce)
        out = (A2*g2 + A0) + t           (affine_then_add)
  - output is written neuron-major [1024, 2048] f16, one write per slot so
    writes overlap the remaining gathers; host reassembles/transposes/upcasts.
"""

import numpy as np

BATCH = 2048
NIN = 8192
NNEUR = 8192
NCORES = 8
NN = NNEUR // NCORES  # neurons per core (1024)
NB = BATCH            # full batch per core
SLOTS = NN // 128     # 8
ACT_SLOT_START = 3    # slots >= this offload their affines to the ACT engine


# gate -> (c0, c1, c2, cp) so gate_k(a1,a2) = c0 + c1*a1 + c2*a2 + cp*a1*a2
GATE_COEF = np.array(
    [
        [0, 0, 0, 0],    # FALSE
        [0, 0, 0, 1],    # AND
        [0, 1, 0, -1],   # a1 AND NOT a2
        [0, 1, 0, 0],    # a1
        [0, 0, 1, -1],   # NOT a1 AND a2
        [0, 0, 1, 0],    # a2
        [0, 1, 1, -2],   # XOR
        [0, 1, 1, -1],   # OR
        [1, -1, -1, 1],  # NOR
        [1, -1, -1, 2],  # XNOR
        [1, 0, -1, 0],   # NOT a2
        [1, 0, -1, 1],   # a1 OR NOT a2
        [1, -1, 0, 0],   # NOT a1
        [1, -1, 0, 1],   # NOT a1 OR a2
        [1, 0, 0, -1],   # NAND
        [1, 0, 0, 0],    # TRUE
    ],
    dtype=np.float32,
)  # [16, 4]

_CACHE = {}


def _build_nc():
    import concourse.bacc as bacc
    import concourse.bass as bass
    import concourse.mybir as mybir
    from concourse.tile import TileContext

    f32 = mybir.dt.float32
    f16 = mybir.dt.float16
    i32 = mybir.dt.int32

    nc = bacc.Bacc("TRN2")
    xt = nc.dram_tensor("xt", [NIN, NB], f16, kind="ExternalInput")
    # io[p, 2*s+o] = row index of operand o for neuron (slot s, partition p)
    io = nc.dram_tensor("io", [128, SLOTS * 2], i32, kind="ExternalInput")
    # ac[p, c, s] = coefficient A_c for neuron (slot s, partition p)
    ac = nc.dram_tensor("ac", [128, 4, SLOTS], f32, kind="ExternalInput")
    yt = nc.dram_tensor("yt", [NN, NB], f16, kind="ExternalOutput")

    with TileContext(nc) as tc:
        with tc.tile_pool(name="all", bufs=1) as pool:
            it = pool.tile([128, SLOTS * 2], i32)
            nc.sync.dma_start(it[:], io[:])
            act = pool.tile([128, 4, SLOTS], f32)
            nc.sync.dma_start(act[:], ac[:])

            # one small tile per slot: the SWDGE descriptor generator mangles
            # dest offsets beyond ~32KB within a single dest AP/tile, so each
            # indirect call must target its own tile at offset ~0.
            # gs[s][p, o, :] = xt[io[p, 2*s+o], :]
            gs = [pool.tile([128, 2, NB], f16, name=f"g{s}") for s in range(SLOTS)]
            uv = [pool.tile([128, 2, NB], f16, name=f"uv{s}") for s in range(SLOTS)]
            ot = pool.tile([128, SLOTS, NB], f16)

            for s in range(SLOTS):
                for o in range(2):
                    j = 2 * s + o
                    nc.gpsimd.indirect_dma_start(
                        out=gs[s][:, o, :], out_offset=None,
                        in_=xt[:],
                        in_offset=bass.IndirectOffsetOnAxis(
                            ap=it[:, j:j + 1], axis=0),
                    )

            # out = (Ap*g2 + A1)*g1 + (A2*g2 + A0)
            #     = (Ap*g1 + A2)*g2 + (A1*g1 + A0)
            # Slots < ACT_SLOT_START run both affines as 4x-packed DVE
            # tensor_scalar ops; the rest offload them to the scalar (ACT)
            # engine so the DVE only does the two 2x-packed tensor_tensor ops.
            for s in range(SLOTS):
                g1 = gs[s][:, 0, :]
                g2 = gs[s][:, 1, :]
                u, v = uv[s][:, 0, :], uv[s][:, 1, :]
                A0 = act[:, 0, s:s + 1]
                A1 = act[:, 1, s:s + 1]
                A2 = act[:, 2, s:s + 1]
                Ap = act[:, 3, s:s + 1]
                if s < ACT_SLOT_START:
                    nc.vector.tensor_scalar(u, g1, Ap, A2,
                                            mybir.AluOpType.mult,
                                            mybir.AluOpType.add)
                    nc.vector.tensor_scalar(v, g1, A1, A0,
                                            mybir.AluOpType.mult,
                                            mybir.AluOpType.add)
                    other = g2
                else:
                    nc.scalar.activation(
                        u, g2, mybir.ActivationFunctionType.Identity,
                        bias=A1, scale=Ap)
                    nc.scalar.activation(
                        v, g2, mybir.ActivationFunctionType.Identity,
                        bias=A0, scale=A2)
                    other = g1
                nc.vector.tensor_mul(ot[:, s, :], u, other)
                nc.vector.tensor_add(ot[:, s, :], ot[:, s, :], v)
                nc.sync.dma_start(yt[s * 128:(s + 1) * 128, :], ot[:, s, :])

    nc.compile()
    return nc


def _prep_core_inputs(x, w, conn_indices):
    """Host-side shard/layout prep. Returns list of per-core input dicts."""
    xt = np.ascontiguousarray(x.T.astype(np.float16))  # [NIN, BATCH] f16, shared
    # A = softmax(w) @ GATE_COEF, [NNEUR, 4] — tiny; compute on host in f64
    ew = np.exp(w.astype(np.float64))
    probs = ew / ew.sum(axis=1, keepdims=True)
    A = (probs @ GATE_COEF.astype(np.float64)).astype(np.float32)
    maps = []
    for c in range(NCORES):
        n0 = c * NN
        # neuron n0 + s*128 + p -> partition p, slot s
        idx = conn_indices[n0:n0 + NN, :].reshape(SLOTS, 128, 2)
        io = idx.transpose(1, 0, 2).reshape(128, SLOTS * 2)
        ac = A[n0:n0 + NN, :].reshape(SLOTS, 128, 4).transpose(1, 2, 0)
        maps.append({
            "xt": xt,
            "io": np.ascontiguousarray(io).astype(np.int32),
            "ac": np.ascontiguousarray(ac),
        })
    return maps


def run_cores(in_maps, trace=False):
    from concourse.bass_utils import run_bass_kernel_spmd

    if "nc" not in _CACHE:
        _CACHE["nc"] = _build_nc()
    return run_bass_kernel_spmd(
        _CACHE["nc"], in_maps, core_ids=list(range(NCORES)), trace=trace
    )


def _assemble(results):
    out = np.empty((BATCH, NNEUR), dtype=np.float32)
    for c in range(NCORES):
        n0 = c * NN
        out[:, n0:n0 + NN] = results[c]["yt"].T.astype(np.float32)
    return out


def kernel(x, w, conn_indices):
    x = np.asarray(x, dtype=np.float32)
    w = np.asarray(w, dtype=np.float32)
    conn_indices = np.asarray(conn_indices)
    in_maps = _prep_core_inputs(x, w, conn_indices)
    res = run_cores(in_maps)
    return _assemble([r for r in res.results])


# revision 12
# speedup vs baseline: 1.5561x; 1.0344x over previous
"""LogicLayer Trainium2 kernel.

out[b, n] = sum_k softmax(w[n])_k * gate_k(a1, a2),  a1 = x[b, i1[n]], a2 = x[b, i2[n]]

All 16 differentiable gates are affine in {1, a1, a2, a1*a2}:
    out[b, n] = A0[n] + A1[n]*a1 + A2[n]*a2 + Ap[n]*a1*a2
with A* = softmax(w[n]) @ C for a constant [16, 4] table C. A* is tiny
([8192, 4] total) and is precomputed on the host, so the device kernel is a
pure gather + 2-DVE-ops-per-slot + write pipeline.

Device plan (8 NeuronCores, neuron-sharded: 1024 neurons x full 2048 batch each):
  - x is shipped transposed and cast to f16 (xt [8192, 2048]) so one neuron's
    input column is a contiguous 4KB row in HBM. The correctness gate is
    rel_err < 2e-2; f16 gathers + f16 output writes land at ~3e-4 while
    halving HBM traffic vs f32 (~12MB/core instead of ~24MB).
  - gathers: 16 single-offset gpsimd.indirect_dma_start calls (multi-offset
    offset-APs and >32KB dest offsets within one dest tile both break the
    real SWDGE descriptor generator, although CoreSim accepts them), each
    into its own small per-slot tile.
  - all working tiles are static f16 SBUF residents (~100KB/partition), so
    there is no pool-buffer recycling and no resulting Q7 stalls.
  - inner loop is 2 fused DVE ops per 128-neuron slot (f16 data hits the
    2x-packed DVE mode):
        t   = (Ap*g2 + A1) * g1          (affine_mul_reduce)
        out = (A2*g2 + A0) + t           (affine_then_add)
  - output is written neuron-major [1024, 2048] f16, one write per slot so
    writes overlap the remaining gathers; host reassembles/transposes/upcasts.
"""

import numpy as np

BATCH = 2048
NIN = 8192
NNEUR = 8192
NCORES = 8
NN = NNEUR // NCORES  # neurons per core (1024)
NB = BATCH            # full batch per core
SLOTS = NN // 128     # 8
ACT_SLOTS = 5         # slots < this offload their affines to the ACT engine;
                      # later slots use DVE tensor_scalar so the tail after the
                      # last gather avoids the cross-engine ACT chain


# gate -> (c0, c1, c2, cp) so gate_k(a1,a2) = c0 + c1*a1 + c2*a2 + cp*a1*a2
GATE_COEF = np.array(
    [
        [0, 0, 0, 0],    # FALSE
        [0, 0, 0, 1],    # AND
        [0, 1, 0, -1],   # a1 AND NOT a2
        [0, 1, 0, 0],    # a1
        [0, 0, 1, -1],   # NOT a1 AND a2
        [0, 0, 1, 0],    # a2
        [0, 1, 1, -2],   # XOR
        [0, 1, 1, -1],   # OR
        [1, -1, -1, 1],  # NOR
        [1, -1, -1, 2],  # XNOR
        [1, 0, -1, 0],   # NOT a2
        [1, 0, -1, 1],   # a1 OR NOT a2
        [1, -1, 0, 0],   # NOT a1
        [1, -1, 0, 1],   # NOT a1 OR a2
        [1, 0, 0, -1],   # NAND
        [1, 0, 0, 0],    # TRUE
    ],
    dtype=np.float32,
)  # [16, 4]

_CACHE = {}


def _build_nc():
    import concourse.bacc as bacc
    import concourse.bass as bass
    import concourse.mybir as mybir
    from concourse.tile import TileContext

    f32 = mybir.dt.float32
    f16 = mybir.dt.float16
    i32 = mybir.dt.int32

    nc = bacc.Bacc("TRN2")
    xt = nc.dram_tensor("xt", [NIN, NB], f16, kind="ExternalInput")
    # io[p, 2*s+o] = row index of operand o for neuron (slot s, partition p)
    io = nc.dram_tensor("io", [128, SLOTS * 2], i32, kind="ExternalInput")
    # ac[p, c, s] = coefficient A_c for neuron (slot s, partition p)
    ac = nc.dram_tensor("ac", [128, 4, SLOTS], f32, kind="ExternalInput")
    yt = nc.dram_tensor("yt", [NN, NB], f16, kind="ExternalOutput")

    with TileContext(nc) as tc:
        with tc.tile_pool(name="all", bufs=1) as pool:
            it = pool.tile([128, SLOTS * 2], i32)
            nc.sync.dma_start(it[:], io[:])
            act = pool.tile([128, 4, SLOTS], f32)
            nc.sync.dma_start(act[:], ac[:])

            # one small tile per slot: the SWDGE descriptor generator mangles
            # dest offsets beyond ~32KB within a single dest AP/tile, so each
            # indirect call must target its own tile at offset ~0.
            # gs[s][p, o, :] = xt[io[p, 2*s+o], :]
            gs = [pool.tile([128, 2, NB], f16, name=f"g{s}") for s in range(SLOTS)]
            uv = [pool.tile([128, 2, NB], f16, name=f"uv{s}") for s in range(SLOTS)]
            ot = pool.tile([128, SLOTS, NB], f16)

            for s in range(SLOTS):
                for o in range(2):
                    j = 2 * s + o
                    nc.gpsimd.indirect_dma_start(
                        out=gs[s][:, o, :], out_offset=None,
                        in_=xt[:],
                        in_offset=bass.IndirectOffsetOnAxis(
                            ap=it[:, j:j + 1], axis=0),
                    )

            # out = (Ap*g2 + A1)*g1 + (A2*g2 + A0)
            #     = (Ap*g1 + A2)*g2 + (A1*g1 + A0)
            # Slots < ACT_SLOT_START run both affines as 4x-packed DVE
            # tensor_scalar ops; the rest offload them to the scalar (ACT)
            # engine so the DVE only does the two 2x-packed tensor_tensor ops.
            for s in range(SLOTS):
                g1 = gs[s][:, 0, :]
                g2 = gs[s][:, 1, :]
                u, v = uv[s][:, 0, :], uv[s][:, 1, :]
                A0 = act[:, 0, s:s + 1]
                A1 = act[:, 1, s:s + 1]
                A2 = act[:, 2, s:s + 1]
                Ap = act[:, 3, s:s + 1]
                if s >= ACT_SLOTS:
                    nc.vector.tensor_scalar(u, g1, Ap, A2,
                                            mybir.AluOpType.mult,
                                            mybir.AluOpType.add)
                    nc.vector.tensor_scalar(v, g1, A1, A0,
                                            mybir.AluOpType.mult,
                                            mybir.AluOpType.add)
                    other = g2
                else:
                    nc.scalar.activation(
                        u, g2, mybir.ActivationFunctionType.Identity,
                        bias=A1, scale=Ap)
                    nc.scalar.activation(
                        v, g2, mybir.ActivationFunctionType.Identity,
                        bias=A0, scale=A2)
                    other = g1
                nc.vector.tensor_mul(ot[:, s, :], u, other)
                nc.vector.tensor_add(ot[:, s, :], ot[:, s, :], v)
                nc.sync.dma_start(yt[s * 128:(s + 1) * 128, :], ot[:, s, :])

    nc.compile()
    return nc


def _prep_core_inputs(x, w, conn_indices):
    """Host-side shard/layout prep. Returns list of per-core input dicts."""
    xt = np.ascontiguousarray(x.T.astype(np.float16))  # [NIN, BATCH] f16, shared
    # A = softmax(w) @ GATE_COEF, [NNEUR, 4] — tiny; compute on host in f64
    ew = np.exp(w.astype(np.float64))
    probs = ew / ew.sum(axis=1, keepdims=True)
    A = (probs @ GATE_COEF.astype(np.float64)).astype(np.float32)
    maps = []
    for c in range(NCORES):
        n0 = c * NN
        # neuron n0 + s*128 + p -> partition p, slot s
        idx = conn_indices[n0:n0 + NN, :].reshape(SLOTS, 128, 2)
        io = idx.transpose(1, 0, 2).reshape(128, SLOTS * 2)
        ac = A[n0:n0 + NN, :].reshape(SLOTS, 128, 4).transpose(1, 2, 0)
        maps.append({
            "xt": xt,
            "io": np.ascontiguousarray(io).astype(np.int32),
            "ac": np.ascontiguousarray(ac),
        })
    return maps


def run_cores(in_maps, trace=False):
    from concourse.bass_utils import run_bass_kernel_spmd

    if "nc" not in _CACHE:
        _CACHE["nc"] = _build_nc()
    return run_bass_kernel_spmd(
        _CACHE["nc"], in_maps, core_ids=list(range(NCORES)), trace=trace
    )


def _assemble(results):
    out = np.empty((BATCH, NNEUR), dtype=np.float32)
    for c in range(NCORES):
        n0 = c * NN
        out[:, n0:n0 + NN] = results[c]["yt"].T.astype(np.float32)
    return out


def kernel(x, w, conn_indices):
    x = np.asarray(x, dtype=np.float32)
    w = np.asarray(w, dtype=np.float32)
    conn_indices = np.asarray(conn_indices)
    in_maps = _prep_core_inputs(x, w, conn_indices)
    res = run_cores(in_maps)
    return _assemble([r for r in res.results])
